# revision 29
# baseline (speedup 1.0000x reference)
"""Trainium2 Bass kernel for nn_CompleteAttention_68418829025814.

Linformer-style windowed attention, restructured for the PE array:
  - window_reverse is folded into a host-side column permutation of E_w/F_w
    (device works entirely in x's native window order) and a host-side
    permutation of the gathered output.
  - k/v are never materialized: k_low = (E @ x) @ Wk^T + const (the E/F
    projections contract over tokens, so x is used in its native layout).
  - only the q path needs x transposed; done on-device via PE transpose mode.
  - all large matmuls run as float32r (full PE rate at moving-dim >= 256);
    the attn@V + softmax-denominator stage uses bf16 col-packed matmuls
    (tile_position col groups are bf16-only), which lands each head's
    denominator partition-aligned with its output for the DVE division.

Sharding: data-parallel over batch; each of the 8 cores gets 4 batches
(256 windows) of x. Small weights are replicated.
"""

import numpy as np

B_TOT = 32
N_CORES = 8
B_PER = B_TOT // N_CORES      # 4 batches per core
N = 3136                      # tokens per batch
NP = 3200                     # padded tokens per batch (6*512 + 128)
C = 192
H = 6
HD = 32
R = 128
WS = 7

_STATE = {}


def _window_perm():
    """n_of_m[m] = spatial index n for window-order position m."""
    hh, ww, i, j = np.meshgrid(
        np.arange(8), np.arange(8), np.arange(7), np.arange(7), indexing="ij"
    )
    m = (hh * 8 + ww) * 49 + i * 7 + j
    n = (hh * 7 + i) * 56 + ww * 7 + j
    n_of_m = np.empty(N, dtype=np.int64)
    n_of_m[m.ravel()] = n.ravel()
    return n_of_m


def _build_bass():
    import concourse.bacc as bacc
    import concourse.mybir as mybir
    from concourse.tile import TileContext

    f32 = mybir.dt.float32
    f32r = mybir.dt.float32r
    f16 = mybir.dt.float16

    nc = bacc.Bacc("TRN2", target_bir_lowering=False, debug=False)

    f8 = mybir.dt.float8e4

    # x_a: phase-A layout, row (p2*NP + n) = [x[2*p2, n, :], x[2*p2+1, n, :]]
    x_d = nc.dram_tensor("x_a", [2 * NP, 2 * C], f16, kind="ExternalInput")
    # xT for the q projection, fp8, contract split into two 96-row halves
    # (DoubleRow layout): row k, half p, col j = x[j, 96*p + k]
    xq_d = nc.dram_tensor("xq8", [96, 2 * B_PER * NP], f8, kind="ExternalInput")
    wq8h_d = nc.dram_tensor("wq8_hi", [96, 2 * 128], f8, kind="ExternalInput")
    wq8l_d = nc.dram_tensor("wq8_lo", [96, 2 * 64], f8, kind="ExternalInput")
    # e/f shipped pre-chunked: row p = 24 chunks of 128 R-values (token 128k+p)
    e_d = nc.dram_tensor("e_wxt", [128, 24 * R], f16, kind="ExternalInput")
    f_d = nc.dram_tensor("f_wxt", [128, 24 * R], f16, kind="ExternalInput")
    e_tl_d = nc.dram_tensor("e_tl", [64, R], f16, kind="ExternalInput")
    f_tl_d = nc.dram_tensor("f_tl", [64, R], f16, kind="ExternalInput")
    bq_d = nc.dram_tensor("bq", [C, 1], f32, kind="ExternalInput")
    wkt_d = nc.dram_tensor("wkt", [C, C], f16, kind="ExternalInput")
    wvt_d = nc.dram_tensor("wvt", [C, C], f16, kind="ExternalInput")
    ckt_d = nc.dram_tensor("const_kt", [C, R], f32, kind="ExternalInput")
    cv_d = nc.dram_tensor("const_v", [R, C], f32, kind="ExternalInput")
    pw_hi_d = nc.dram_tensor("projwt_hi", [128, C], f16, kind="ExternalInput")
    pw_lo_d = nc.dram_tensor("projwt_lo_aug", [65, C], f16, kind="ExternalInput")
    ident_d = nc.dram_tensor("ident", [128, 128], f16, kind="ExternalInput")
    ones_d = nc.dram_tensor("ones_att", [128, 32], f16, kind="ExternalInput")
    onesrow_d = nc.dram_tensor("ones_row", [1, 512], f16, kind="ExternalInput")
    out_d = nc.dram_tensor("out", [B_PER * NP, C], f16, kind="ExternalOutput")

    NCH = 25  # n-chunks per batch for the E/F contraction (24*128 + 64)

    with TileContext(nc) as tc:
        with tc.tile_pool(name="const", bufs=1) as cpool, \
             tc.tile_pool(name="ef", bufs=1) as efpool, \
             tc.tile_pool(name="low", bufs=1) as lowpool, \
             tc.tile_pool(name="xin", bufs=6) as xpool, \
             tc.tile_pool(name="xt", bufs=3) as xtpool, \
             tc.tile_pool(name="qt", bufs=2) as qtpool, \
             tc.tile_pool(name="sp", bufs=2) as sppool, \
             tc.tile_pool(name="div", bufs=2) as divpool, \
             tc.tile_pool(name="av", bufs=2) as avpool, \
             tc.tile_pool(name="osb", bufs=4) as opool, \
             tc.tile_pool(name="ps", bufs=8, space="PSUM") as ps:

            # ---- constants ----
            ident = cpool.tile([128, 128], f16)
            nc.scalar.dma_start(ident[:], ident_d[:])
            wq8h = cpool.tile([96, 2, 128], f8)
            nc.scalar.dma_start(
                wq8h[:], wq8h_d[:].rearrange("p (two m) -> p two m", two=2)
            )
            wq8l = cpool.tile([96, 2, 64], f8)
            nc.scalar.dma_start(
                wq8l[:], wq8l_d[:].rearrange("p (two m) -> p two m", two=2)
            )
            bq_h = cpool.tile([128, 1], f32)
            nc.scalar.dma_start(bq_h[:], bq_d[0:128, :])
            bq_l = cpool.tile([64, 1], f32)
            nc.scalar.dma_start(bq_l[:], bq_d[128:192, :])
            wkt = cpool.tile([128, C], f16)
            nc.scalar.dma_start(wkt[:], wkt_d[0:128, :])
            wkt_l = cpool.tile([64, C], f16)
            nc.scalar.dma_start(wkt_l[:], wkt_d[128:192, :])
            wvt = cpool.tile([128, C], f16)
            nc.scalar.dma_start(wvt[:], wvt_d[0:128, :])
            wvt_l = cpool.tile([64, C], f16)
            nc.scalar.dma_start(wvt_l[:], wvt_d[128:192, :])
            ckt_h = cpool.tile([128, R], f32)
            nc.scalar.dma_start(ckt_h[:], ckt_d[0:128, :])
            ckt_l = cpool.tile([64, R], f32)
            nc.scalar.dma_start(ckt_l[:], ckt_d[128:192, :])
            cv = cpool.tile([128, C], f32)
            nc.scalar.dma_start(cv[:], cv_d[:])
            pw_hi = cpool.tile([128, C], f16)
            nc.scalar.dma_start(pw_hi[:], pw_hi_d[:])
            pw_lo = cpool.tile([65, C], f16)
            nc.scalar.dma_start(pw_lo[:], pw_lo_d[:])
            ones_att = cpool.tile([128, 32], f16)
            nc.scalar.dma_start(ones_att[:], ones_d[:])
            # two persistent [65, 512] attn-output staging tiles whose row 64
            # stays 1.0 forever (feeds proj_b through pw_lo's last row)
            av_lo_bufs = [cpool.tile([65, 512], f16, name=f"avlo{i}") for i in range(2)]
            for i in range(2):
                nc.scalar.dma_start(av_lo_bufs[i][64:65, :], onesrow_d[:])

            # E/F transposed weights resident in SBUF: 24 full chunks + tail
            # (shipped pre-chunked so each partition line is one contiguous
            # DMA descriptor). Loaded in 4 groups of 6 chunks, interleaved
            # with the first x2 stream so EP compute starts early.
            e_sb = efpool.tile([128, 24, 128], f16)
            f_sb = efpool.tile([128, 24, 128], f16)
            e_tl = efpool.tile([64, 128], f16)
            f_tl = efpool.tile([64, 128], f16)

            def load_ef_group(g):
                sl = slice(g * 6, (g + 1) * 6)
                dsl = slice(g * 6 * 128, (g + 1) * 6 * 128)
                nc.sync.dma_start(
                    e_sb[:, sl, :].rearrange("p k r -> p (k r)"), e_d[:, dsl]
                )
                nc.sync.dma_start(
                    f_sb[:, sl, :].rearrange("p k r -> p (k r)"), f_d[:, dsl]
                )

            load_ef_group(0)

            # per-batch low-rank tensors (kept resident across phase B)
            klo_h = [lowpool.tile([128, R], f16, name=f"klo_h{b}") for b in range(B_PER)]
            klo_l = [lowpool.tile([64, R], f16, name=f"klo_l{b}") for b in range(B_PER)]
            vlo = [lowpool.tile([128, C], f16, name=f"vlo{b}") for b in range(B_PER)]

            # ---------------- Phase A: EP/FP + low-rank projections ----------
            for p2 in range(2):
                ep_ps = ps.tile([128, 2 * C], f32, name="ep_ps", tag="bank")
                fp_ps = ps.tile([128, 2 * C], f32, name="fp_ps", tag="bank")
                for ci in range(NCH):
                    nk = 128 if ci < 24 else 64
                    x2 = xpool.tile([nk, 2 * C], f16, name="x2", tag="x2")
                    nc.sync.dma_start(
                        x2[:],
                        x_d[p2 * NP + ci * 128 : p2 * NP + ci * 128 + nk, :],
                    )
                    if p2 == 0 and ci in (1, 6, 11):
                        load_ef_group(ci // 5 + 1)
                    if p2 == 0 and ci == 16:
                        nc.sync.dma_start(e_tl[:], e_tl_d[:])
                        nc.sync.dma_start(f_tl[:], f_tl_d[:])
                    elh = e_sb[:, ci, :] if ci < 24 else e_tl[:]
                    flh = f_sb[:, ci, :] if ci < 24 else f_tl[:]
                    x2f = x2[:]
                    nc.tensor.matmul(
                        ep_ps[:], elh, x2f, start=(ci == 0), stop=(ci == NCH - 1)
                    )
                    nc.tensor.matmul(
                        fp_ps[:], flh, x2f, start=(ci == 0), stop=(ci == NCH - 1)
                    )
                ep_sb = xpool.tile([128, 2 * C], f16, name="ep_sb", tag="ep_sb")
                nc.vector.tensor_copy(ep_sb[:], ep_ps[:])
                fp_sb = xpool.tile([128, 2 * C], f16, name="fp_sb", tag="fp_sb")
                nc.vector.tensor_copy(fp_sb[:], fp_ps[:])

                for b2 in range(2):
                    b = 2 * p2 + b2
                    # transpose EP, FP slices: (r=128, c=192) -> (c, r)
                    ept_h = xpool.tile([128, 128], f16, name="ept_h", tag="ept_h")
                    ept_l = xpool.tile([64, 128], f16, name="ept_l", tag="ept_l")
                    fpt_h = xpool.tile([128, 128], f16, name="fpt_h", tag="fpt_h")
                    fpt_l = xpool.tile([64, 128], f16, name="fpt_l", tag="fpt_l")
                    for (src, dsth, dstl) in ((ep_sb, ept_h, ept_l), (fp_sb, fpt_h, fpt_l)):
                        tp1 = ps.tile([128, 128], f16, name="tp1", tag="bank")
                        nc.tensor.transpose(
                            tp1[:], src[:, b2 * C : b2 * C + 128], ident[:]
                        )
                        nc.vector.tensor_copy(dsth[:], tp1[:])
                        tp2 = ps.tile([64, 128], f16, name="tp2", tag="bank")
                        nc.tensor.transpose(
                            tp2[:], src[:, b2 * C + 128 : b2 * C + 192], ident[:]
                        )
                        nc.vector.tensor_copy(dstl[:], tp2[:])

                    # k_lowT = WkT.T @ EPT + const_kT  (feature-major (kch, r))
                    kl_hi = ps.tile([128, R], f32, name="kl_hi", tag="bank")
                    nc.tensor.matmul(kl_hi[:], wkt[:, 0:128], ept_h[:], start=True, stop=False)
                    nc.tensor.matmul(kl_hi[:], wkt_l[:, 0:128], ept_l[:], start=False, stop=True)
                    nc.vector.tensor_tensor(
                        klo_h[b][:], kl_hi[:], ckt_h[:], op=mybir.AluOpType.add
                    )
                    kl_lo = ps.tile([64, R], f32, name="kl_lo", tag="bank")
                    nc.tensor.matmul(kl_lo[:], wkt[:, 128:192], ept_h[:], start=True, stop=False)
                    nc.tensor.matmul(kl_lo[:], wkt_l[:, 128:192], ept_l[:], start=False, stop=True)
                    nc.vector.tensor_tensor(
                        klo_l[b][:], kl_lo[:], ckt_l[:], op=mybir.AluOpType.add
                    )
                    # v_low (R-major (r, vch)), straight to bf16 with const add
                    vl_ps = ps.tile([128, C], f32, name="vl_ps", tag="bank")
                    nc.tensor.matmul(vl_ps[:], fpt_h[:], wvt[:], start=True, stop=False)
                    nc.tensor.matmul(vl_ps[:], fpt_l[:], wvt_l[:], start=False, stop=True)
                    nc.vector.tensor_tensor(
                        vlo[b][:], vl_ps[:], cv[:], op=mybir.AluOpType.add
                    )

            # -------- Phase A2: q projection for all tiles (dense PE pass) ----
            # fp8 DoubleRow: contract 192 split as [96, 2]; weights carry a
            # 2^7 gain (e4m3 subnormal avoidance) undone in the copy-out.
            qth = [qtpool.tile([128, NP], f16, name=f"qth{b}") for b in range(B_PER)]
            qtl = [qtpool.tile([64, NP], f16, name=f"qtl{b}") for b in range(B_PER)]
            xq_v = xq_d[:].rearrange("p (two m) -> p two m", two=2)
            for b in range(B_PER):
                for t in range(7):
                    W = 512 if t < 6 else 128
                    KCH = W // 128
                    base = b * NP + t * 512
                    xq = xtpool.tile([96, 2, W], f8, name="xq", tag="xq")
                    nc.scalar.dma_start(xq[:], xq_v[:, :, base : base + W])
                    q_hi = ps.tile([128, W], f32, name="q_hi", tag="bank")
                    nc.tensor.matmul(
                        q_hi[:], wq8h[:], xq[:], start=True, stop=True,
                        perf_mode=mybir.MatmulPerfMode.DoubleRow,
                    )
                    q_lo = ps.tile([64, W], f32, name="q_lo", tag="bank")
                    nc.tensor.matmul(
                        q_lo[:], wq8l[:], xq[:], start=True, stop=True,
                        perf_mode=mybir.MatmulPerfMode.DoubleRow,
                    )
                    nc.scalar.activation(
                        qth[b][:, t * 512 : t * 512 + W], q_hi[:],
                        mybir.ActivationFunctionType.Identity,
                        bias=bq_h[:], scale=2.0 ** -7,
                    )
                    nc.vector.tensor_scalar(
                        out=qtl[b][:, t * 512 : t * 512 + W], in0=q_lo[:],
                        scalar1=2.0 ** -7, scalar2=bq_l[:],
                        op0=mybir.AluOpType.mult, op1=mybir.AluOpType.add,
                    )

            # ---------------- Phase B: attention tiles ----------
            def front(b, t):
                W = 512 if t < 6 else 128
                KCH = W // 128
                base = b * NP + t * 512
                tok = t * 512
                # scores (f16 row-packed) + exp, head by head
                spt = []
                for h in range(H):
                    s_ps = ps.tile([128, W], f32, name=f"s{h}", tag="bank")
                    if h < 4:
                        nc.tensor.matmul(
                            s_ps[:],
                            klo_h[b][32 * h : 32 * h + 32, :],
                            qth[b][32 * h : 32 * h + 32, tok : tok + W],
                            start=True, stop=True,
                            tile_position=(32 * h, 0),
                        )
                    else:
                        hh = h - 4
                        nc.tensor.matmul(
                            s_ps[:],
                            klo_l[b][32 * hh : 32 * hh + 32, :],
                            qtl[b][32 * hh : 32 * hh + 32, tok : tok + W],
                            start=True, stop=True,
                            tile_position=(32 * hh, 0),
                        )
                    sp_t = sppool.tile([128, W], f16, name=f"sp{h}", tag=f"sp{h}")
                    nc.scalar.activation(
                        sp_t[:], s_ps[:], mybir.ActivationFunctionType.Exp
                    )
                    spt.append(sp_t)

                # attn @ v_low + denominators (f16 col-packed). All 4 low
                # heads share one av psum tile and one z psum tile (disjoint
                # 32-partition bands). Grouped issue: the 4 av stationaries
                # occupy 4 distinct col bands and stay resident, so the 4
                # streams overlap (same trick the score row-bands use); the
                # z group then reloads the bands with ones.
                avA = ps.tile([128, W], f32, name="avA", tag="bank")
                for h in range(4):
                    nc.tensor.matmul(
                        avA[32 * h : 32 * h + 32, :],
                        vlo[b][:, 32 * h : 32 * h + 32],
                        spt[h][:],
                        start=True, stop=True,
                        tile_position=(0, 32 * h),
                    )
                zA = ps.tile([128, W], f32, name="zA", tag="bank")
                for h in range(4):
                    nc.tensor.matmul(
                        zA[32 * h : 32 * h + 32, :],
                        ones_att[:],
                        spt[h][:],
                        start=True, stop=True,
                        tile_position=(0, 32 * h),
                    )
                av2 = ps.tile([64, W], f32, name="av2", tag="bank")
                z2 = ps.tile([64, W], f32, name="z2", tag="bank")
                for h in range(4, 6):
                    hh = h - 4
                    nc.tensor.matmul(
                        av2[32 * hh : 32 * hh + 32, :],
                        vlo[b][:, 32 * h : 32 * h + 32],
                        spt[h][:],
                        start=True, stop=True,
                        tile_position=(0, 32 * hh),
                    )
                for h in range(4, 6):
                    hh = h - 4
                    nc.tensor.matmul(
                        z2[32 * hh : 32 * hh + 32, :],
                        ones_att[:],
                        spt[h][:],
                        start=True, stop=True,
                        tile_position=(0, 32 * hh),
                    )
                av_hi = avpool.tile([128, W], f16, name="av_hi", tag="av_hi")
                av_lo = av_lo_bufs[(b * 7 + t) % 2]
                return dict(
                    W=W, KCH=KCH, base=base, avA=avA, zA=zA,
                    av2=av2, z2=z2, av_hi=av_hi, av_lo=av_lo,
                )

            def back(st):
                W, KCH, base = st["W"], st["KCH"], st["base"]
                rzA = divpool.tile([128, W], f32, name="rzA", tag="rzA")
                nc.vector.reciprocal_approx_fast(rzA[:], st["zA"][:])
                rz2 = divpool.tile([64, W], f32, name="rz2", tag="rz2")
                nc.vector.reciprocal_approx_fast(rz2[:], st["z2"][:])
                av_hi, av_lo = st["av_hi"], st["av_lo"]
                nc.vector.tensor_tensor(
                    av_hi[:, :], st["avA"][:], rzA[:], op=mybir.AluOpType.mult
                )
                nc.vector.tensor_tensor(
                    av_lo[0:64, 0:W], st["av2"][:], rz2[:], op=mybir.AluOpType.mult
                )
                o_sb = opool.tile([128, KCH, C], f16, name="o_sb", tag="o_sb")
                for m in range(KCH):
                    p_ps = ps.tile([128, C], f32, name="p_ps", tag="bank")
                    nc.tensor.matmul(
                        p_ps[:],
                        av_hi[:, m * 128 : (m + 1) * 128],
                        pw_hi[:],
                        start=True, stop=False,
                    )
                    nc.tensor.matmul(
                        p_ps[:],
                        av_lo[:, m * 128 : (m + 1) * 128],
                        pw_lo[:],
                        start=False, stop=True,
                    )
                    if m % 2 == 0:
                        nc.scalar.copy(o_sb[:, m, :], p_ps[:])
                    else:
                        nc.vector.tensor_copy(o_sb[:, m, :], p_ps[:])
                nc.gpsimd.dma_start(
                    out_d[base : base + W, :].rearrange("(m p) c -> p m c", p=128),
                    o_sb[:],
                )

            tiles = [(b, t) for b in range(B_PER) for t in range(7)]
            prev = None
            for (b, t) in tiles:
                st = front(b, t)
                if prev is not None:
                    back(prev)
                prev = st
            back(prev)

    nc.compile()
    return nc


def _get_nc():
    if "nc" not in _STATE:
        _STATE["nc"] = _build_bass()
    return _STATE["nc"]


def kernel(x, qkv_w, qkv_b, E_w, E_b, F_w, F_b, proj_w, proj_b, h, w):
    from concourse.bass_utils import run_bass_kernel_spmd

    x = np.asarray(x, dtype=np.float32)
    qkv_w = np.asarray(qkv_w, dtype=np.float32)
    qkv_b = np.asarray(qkv_b, dtype=np.float32)
    E_w = np.asarray(E_w, dtype=np.float32)
    E_b = np.asarray(E_b, dtype=np.float32)
    F_w = np.asarray(F_w, dtype=np.float32)
    F_b = np.asarray(F_b, dtype=np.float32)
    proj_w = np.asarray(proj_w, dtype=np.float32)
    proj_b = np.asarray(proj_b, dtype=np.float32)
    assert int(h) == 56 and int(w) == 56

    n_of_m = _window_perm()
    E_wx = np.ascontiguousarray(E_w[:, n_of_m])
    F_wx = np.ascontiguousarray(F_w[:, n_of_m])

    Wq, Wk, Wv = qkv_w[0:C], qkv_w[C : 2 * C], qkv_w[2 * C : 3 * C]
    bq, bk, bv = qkv_b[0:C], qkv_b[C : 2 * C], qkv_b[2 * C : 3 * C]
    scale = np.float32(1.0 / np.sqrt(HD))

    const_k = np.outer(E_wx.sum(1), bk) + E_b[:, None]      # (128, 192)
    const_v = (np.outer(F_wx.sum(1), bv) + F_b[:, None]).astype(np.float32)

    import ml_dtypes

    f8np = ml_dtypes.float8_e4m3
    # q weights in fp8 DoubleRow layout [96, 2, m] with a 2^7 gain
    wqs_t = np.ascontiguousarray((Wq * scale).T * 128.0)      # (c, m)
    wq_dr = wqs_t.reshape(2, 96, C).transpose(1, 0, 2)        # (96, 2, m)
    wq8_hi = np.ascontiguousarray(wq_dr[:, :, 0:128]).reshape(96, 256).astype(f8np)
    wq8_lo = np.ascontiguousarray(wq_dr[:, :, 128:192]).reshape(96, 128).astype(f8np)
    bq_s = np.ascontiguousarray((bq * scale).reshape(C, 1))
    wkt = np.ascontiguousarray(Wk.T).astype(np.float16)
    wvt = np.ascontiguousarray(Wv.T).astype(np.float16)
    ckt = np.ascontiguousarray(const_k.T.astype(np.float32))  # (192, 128)
    pw = proj_w.T                                            # (ch, co)
    pw_hi = np.ascontiguousarray(pw[0:128]).astype(np.float16)
    pw_lo = np.zeros((65, C), dtype=np.float16)
    pw_lo[0:64] = pw[128:192]
    pw_lo[64] = proj_b

    e_wxt_full = np.ascontiguousarray(E_wx.T).astype(np.float16)  # (3136, 128)
    f_wxt_full = np.ascontiguousarray(F_wx.T).astype(np.float16)
    # pre-chunked layout: (24, 128, R) -> (128, 24*R) so each SBUF partition
    # line is one contiguous DMA descriptor
    e_wxt = np.ascontiguousarray(
        e_wxt_full[0:3072].reshape(24, 128, R).transpose(1, 0, 2).reshape(128, 24 * R)
    )
    f_wxt = np.ascontiguousarray(
        f_wxt_full[0:3072].reshape(24, 128, R).transpose(1, 0, 2).reshape(128, 24 * R)
    )
    e_tl = np.ascontiguousarray(e_wxt_full[3072:3136])
    f_tl = np.ascontiguousarray(f_wxt_full[3072:3136])
    ident = np.eye(128, dtype=np.float16)
    ones_att = np.ones((128, 32), dtype=np.float16)
    ones_row = np.ones((1, 512), dtype=np.float16)

    consts = dict(
        e_wxt=e_wxt, f_wxt=f_wxt, e_tl=e_tl, f_tl=f_tl,
        wq8_hi=wq8_hi, wq8_lo=wq8_lo, bq=bq_s, wkt=wkt, wvt=wvt,
        const_kt=ckt, const_v=const_v, projwt_hi=pw_hi, projwt_lo_aug=pw_lo,
        ident=ident, ones_att=ones_att, ones_row=ones_row,
    )

    # shard x: core i gets batches 4i..4i+4, padded to NP tokens per batch
    xb = x.reshape(B_TOT, 64 * 49, C).astype(np.float16)
    in_maps = []
    for i in range(N_CORES):
        xi = np.zeros((B_PER, NP, C), dtype=np.float16)
        xi[:, 0:N, :] = xb[B_PER * i : B_PER * (i + 1)]
        # phase-A layout: (p2, n, pair*C) so chunk loads are fully contiguous
        xa = np.ascontiguousarray(
            xi.reshape(2, 2, NP, C).transpose(0, 2, 1, 3).reshape(2 * NP, 2 * C)
        )
        M = B_PER * NP
        xt = xi.reshape(M, C).T                               # (c, M)
        xq8 = np.ascontiguousarray(
            xt.reshape(2, 96, M).transpose(1, 0, 2).reshape(96, 2 * M)
        ).astype(f8np)
        in_maps.append({**consts, "x_a": xa, "xq8": xq8})

    nc = _get_nc()
    _STATE["last_in_maps"] = in_maps
    res = run_bass_kernel_spmd(nc, in_maps, core_ids=list(range(N_CORES)))

    out_win = np.empty((B_TOT, N, C), dtype=np.float32)
    for i in range(N_CORES):
        oi = res.results[i]["out"].astype(np.float32).reshape(B_PER, NP, C)
        out_win[B_PER * i : B_PER * (i + 1)] = oi[:, 0:N, :]
    # window_reverse on the gathered output
    out_sp = (
        out_win.reshape(B_TOT, 8, 8, 7, 7, C)
        .transpose(0, 1, 3, 2, 4, 5)
        .reshape(B_TOT, N, C)
    )
    return np.ascontiguousarray(out_sp)

